# revision 20
# baseline (speedup 1.0000x reference)
"""Cross-attention head (B=4, T=S=4096, C=1024, HS=64) on 8 TRN2 NeuronCores.

Sharding: core i handles batch b = i//2, query-half th = i%2 (2048 query rows).
Each core gets a transposed slice xT [C, 2048] and its batch's encT [C, S]
(host-side layout prep, cast to fp16), plus packed weights Wqq=[Wq|Wq],
Wkv=[Wk|Wv], Wvk=[Wv|Wk] ([C,128] each, fp16).

All PE matmuls run on fp16 operands (1 cycle/row vs 4 for fp32); PSUM
accumulation stays fp32.  Per-core pipeline:
  qT2 [128, 2048] = (Wqq)^T @ xT           rows 0:64 = q^T, rows 64:128 = copy
  stream over s in 512-chunks, alternating Wkv / Wvk so that k^T lands on
  partitions 0:64 (even chunks) or 64:128 (odd chunks); v^T on the other half.
  v^T chunks are transposed on the PE (identity matmul) into va [128s, 65]
  tiles (col 64 = 1.0, giving the softmax denominator for free).
  scoresT [s,t] = kT^T_block @ qT2: two row-tiled matmuls (PE rows 0:63 and
  64:127) run concurrently; one ACT Exp (scale=1/8) evacuates both PSUM banks
  into a fp16 U tile.
  PV: po[65, t] += va^T @ U accumulated over all 32 s-blocks per t-chunk.
  Tail: po -> fp16 -> transpose -> divide rows by Z (col 64) -> out [2048, 64].
"""

import numpy as np

B, T, S, C, HS = 4, 4096, 4096, 1024, 64
NCORE = 8
TSH = T // 2            # 2048 query rows per core
KT = C // 128           # 8 contraction k-tiles
NTCH = TSH // 512       # 4 t-chunks
NCP = S // 1024         # 4 s-chunk pairs (each pair = 2x 512 keys)
SCALE = HS ** -0.5
import os
OFFLOAD = os.environ.get("K_OFFLOAD", "1") == "1"
POOLCOPY = os.environ.get("K_POOLCOPY", "0") == "1"
# Schraudolph fp16 exp for the DVE-offloaded score tiles:
#   i16 = round(raw_score * EXPA + EXPB);  bitcast(i16) ~= exp(raw_score*SCALE)
# EXPB's -62 mantissa-LSB bias centers the piecewise-linear 2^f error for the
# mixed exact/approx softmax (fitted numerically; ~1.1% output error at 25%
# offload).
EXPA = float(SCALE * np.log2(np.e) * 1024.0)
EXPB = float(15 * 1024 - 62)

_CACHE = {}


def _build(reps=1):
    import concourse.bass as bass
    import concourse.mybir as mybir
    from concourse import bacc
    from concourse.tile import TileContext
    from concourse.masks import make_identity

    f32 = mybir.dt.float32
    f16 = mybir.dt.float16
    i16 = mybir.dt.int16
    Exp = mybir.ActivationFunctionType.Exp
    Mult = mybir.AluOpType.mult
    Add = mybir.AluOpType.add

    nc = bacc.Bacc("TRN2", target_bir_lowering=False, debug=False,
                   num_devices=NCORE)
    # Host-packed fp16 layouts: every DMA sees per-partition contiguous runs.
    # xT packed as [tch, p, k, 512], encT as [sch, p, k, 512],
    # weights as [p, k, 128], out as [tch, p, j, 64] (fp32).
    xT = nc.dram_tensor("xT", [NTCH, 128, KT, 512], f16, kind="ExternalInput")
    encT = nc.dram_tensor("encT", [S // 512, 128, KT, 512], f16,
                          kind="ExternalInput")
    wqq = nc.dram_tensor("Wqq", [128, KT, 128], f16, kind="ExternalInput")
    wkv = nc.dram_tensor("Wkv", [128, KT, 128], f16, kind="ExternalInput")
    wvk = nc.dram_tensor("Wvk", [128, KT, 128], f16, kind="ExternalInput")
    out = nc.dram_tensor("out", [NTCH, 128, 4, HS], f32,
                         kind="ExternalOutput")

    xT_v = xT[:]       # [4, 128, 8, 512]
    encT_v = encT[:]   # [8, 128, 8, 512]
    out_v = out[:]     # [4, 128, 4, 64]

    with TileContext(nc) as tc:
        from contextlib import ExitStack
        with ExitStack() as ctx:
            ep = ctx.enter_context
            wpool = ep(tc.tile_pool(name="w", bufs=1))
            qpool = ep(tc.tile_pool(name="qt", bufs=1))
            xtp = ep(tc.tile_pool(name="xt", bufs=3))
            encp = ep(tc.tile_pool(name="enc", bufs=4))
            kvp = ep(tc.tile_pool(name="kv", bufs=4))
            vap = ep(tc.tile_pool(name="va", bufs=4))
            up = ep(tc.tile_pool(name="u", bufs=4))
            otp = ep(tc.tile_pool(name="ot", bufs=2))
            obp = ep(tc.tile_pool(name="ob", bufs=2))
            rp = ep(tc.tile_pool(name="r", bufs=2))
            # PSUM: po 4 banks + shared transient pool 4x[128,512] = 4 -> 8
            pop = ep(tc.tile_pool(name="po", bufs=1, space="PSUM"))
            psp = ep(tc.tile_pool(name="ps", bufs=4, space="PSUM"))

            # static tiles
            ident = wpool.tile([128, 128], f16, tag="ident")
            make_identity(nc, ident[:])
            w_sb = {}
            for name, dram in (("qq", wqq), ("kv", wkv), ("vk", wvk)):
                wt = wpool.tile([128, KT * 128], f16, tag=f"w{name}")
                nc.sync.dma_start(
                    out=wt[:].rearrange("p (k m) -> p k m", k=KT),
                    in_=dram[:])
                w_sb[name] = wt[:].rearrange("p (k m) -> p k m", k=KT)

            for _rep in range(reps):
                qt2 = qpool.tile([128, TSH], f16, tag="qt2")

                # HWDGE costs ~0.6us per dma_start, so steady-state transfers
                # stay whole-chunk; only the cold-start tiles (x tch0, enc
                # chunk 0) are split per k so the first matmuls start after
                # ~1/8 of the transfer (HWDGE is idle during warmup anyway).
                def enc_dma(cp, par, split=False):
                    def f():
                        sch = 2 * cp + par
                        enc = encp.tile([128, KT * 512], f16, tag="enc")
                        enc3 = enc[:].rearrange("p (k n) -> p k n", k=KT)
                        if split:
                            for k in range(KT):
                                nc.sync.dma_start(out=enc3[:, k, :],
                                                  in_=encT_v[sch, :, k, :])
                        else:
                            nc.sync.dma_start(out=enc3, in_=encT_v[sch])
                        return enc3
                    return f

                xts = []
                for tch in range(NTCH):
                    xt = xtp.tile([128, KT * 512], f16, tag="xt")
                    xt3 = xt[:].rearrange("p (k n) -> p k n", k=KT)
                    if tch == 0:
                        # interleave split x0 with split enc0-A so the Q-proj
                        # and the kv pipeline both start early
                        enc0a = encp.tile([128, KT * 512], f16, tag="enc")
                        enc0a3 = enc0a[:].rearrange("p (k n) -> p k n", k=KT)
                        for k in range(KT):
                            nc.sync.dma_start(out=xt3[:, k, :],
                                              in_=xT_v[tch, :, k, :])
                            nc.sync.dma_start(out=enc0a3[:, k, :],
                                              in_=encT_v[0, :, k, :])
                    else:
                        nc.sync.dma_start(out=xt3, in_=xT_v[tch])
                    xts.append(xt3)
                enc_pre = [enc0a3, enc_dma(0, 1)()]

                # ---- Phase Q: qT2 = [Wq|Wq]^T @ xT
                for tch in range(NTCH):
                    pq = psp.tile([128, 512], f32, tag="ps", name="pq")
                    for k in range(KT):
                        nc.tensor.matmul(pq[:], w_sb["qq"][:, k, :],
                                         xts[tch][:, k, :],
                                         start=(k == 0), stop=(k == KT - 1))
                    nc.scalar.copy(
                        qt2[:, tch * 512:(tch + 1) * 512], pq[:])

                # ---- Phase S: stream s-chunk pairs
                po = [pop.tile([128, 512], f32, tag=f"po{t}", name=f"po{t}")
                      for t in range(NTCH)]

                def make_kv_thunks(cp, pre=None):
                    """Emit-later closures for loading/projecting s-chunk pair
                    cp.  Returns (thunks, kv_tiles, va_views)."""
                    kvs, vas = [None, None], [None, None]
                    encs = list(pre) if pre else [None, None]
                    thunks = []

                    def dma(par):
                        def f():
                            encs[par] = enc_dma(cp, par)()
                        return f

                    def proj(par):
                        def f():
                            enc3 = encs[par]
                            pkv = psp.tile([128, 512], f32, tag="ps",
                                           name="pkv")
                            wname = "kv" if par == 0 else "vk"
                            for k in range(KT):
                                nc.tensor.matmul(pkv[:], w_sb[wname][:, k, :],
                                                 enc3[:, k, :],
                                                 start=(k == 0),
                                                 stop=(k == KT - 1))
                            kv = kvp.tile([128, 512], f16, tag="kv")
                            nc.scalar.copy(kv[:], pkv[:])
                            va = vap.tile([128, 4 * 65], f16, tag="va")
                            va3 = va[:].rearrange("p (j m) -> p j m", j=4)
                            nc.gpsimd.memset(va3[:, :, 64:65], 1.0)
                            kvs[par] = kv
                            vas[par] = va3
                        return f

                    def vtrans(par, j):
                        def f():
                            va3 = vas[par]
                            vt = kvs[par]
                            rows = slice(64, 128) if par == 0 else slice(0, 64)
                            pvt = psp.tile([128, 65], f32, tag="ps",
                                           name="pvt")
                            nc.tensor.matmul(pvt[:, 0:64],
                                             vt[rows, j * 128:(j + 1) * 128],
                                             ident[rows, rows],
                                             start=True, stop=True)
                            nc.vector.tensor_copy(va3[:, j, 0:64],
                                                  pvt[:, 0:64])
                        return f

                    for par in range(2):
                        if pre is None:
                            thunks.append(dma(par))
                    for par in range(2):
                        thunks.append(proj(par))
                        for j in range(4):
                            thunks.append(vtrans(par, j))
                    return thunks, kvs, vas

                def emit_pv(prev):
                    """PV matmuls for a previously-exp'd pair (one-pair SW
                    pipeline keeps the PE from stalling on the current exp)."""
                    u, pvas, pcp, ptch, ppb = prev
                    first = (pcp == 0 and ppb == 0)
                    last = (pcp == NCP - 1 and ppb == 3)
                    nc.tensor.matmul(po[ptch][0:65, :],
                                     pvas[0][:, ppb, :], u[:, 0:512],
                                     start=first, stop=False,
                                     skip_group_check=True)
                    nc.tensor.matmul(po[ptch][0:65, :],
                                     pvas[1][:, ppb, :], u[:, 512:1024],
                                     start=False, stop=last,
                                     skip_group_check=True)

                def tail(tch):
                    """Normalize po[tch] and store (emitted right after that
                    tch's last PV so it overlaps the remaining steady loop)."""
                    ot = otp.tile([128, 512], f16, tag="ot")
                    nc.vector.tensor_copy(ot[0:65, :], po[tch][0:65, :])
                    ob = obp.tile([128, 4 * 64], f32, tag="ob")
                    ob3 = ob[:].rearrange("p (j d) -> p j d", j=4)
                    for j in range(4):
                        pt = psp.tile([128, 65], f32, tag="ps", name="pt")
                        nc.tensor.matmul(pt[:],
                                         ot[0:65, j * 128:(j + 1) * 128],
                                         ident[0:65, 0:65],
                                         start=True, stop=True)
                        r = rp.tile([128, 1], f32, tag="r")
                        nc.vector.reciprocal(r[:], pt[:, 64:65])
                        nc.vector.tensor_scalar_mul(ob3[:, j, :],
                                                    pt[:, 0:64], r[:])
                    nc.sync.dma_start(out=out_v[tch], in_=ob3)

                cur = make_kv_thunks(0, pre=enc_pre)
                for th in cur[0]:
                    th()
                pending = []   # 2-deep PV software pipeline
                for cp in range(NCP):
                    _, kvs, vas = cur
                    nxt = make_kv_thunks(cp + 1) if cp + 1 < NCP else ([], None, None)
                    n_thunks = len(nxt[0])
                    ti = 0
                    pair_idx = 0
                    for tch in range(NTCH):
                        for pb in range(4):
                            psa = psp.tile([128, 512], f32, tag="ps",
                                           name="psa")
                            psb = psp.tile([128, 512], f32, tag="ps",
                                           name="psb")
                            nc.tensor.matmul(
                                psa[:],
                                kvs[0][0:64, pb * 128:(pb + 1) * 128],
                                qt2[0:64, tch * 512:(tch + 1) * 512],
                                start=True, stop=True)
                            nc.tensor.matmul(
                                psb[:],
                                kvs[1][64:128, pb * 128:(pb + 1) * 128],
                                qt2[64:128, tch * 512:(tch + 1) * 512],
                                start=True, stop=True)
                            if pb == 1 and OFFLOAD:
                                # DVE offload: fp16 Schraudolph exp bit-trick
                                ui = up.tile([128, 1024], i16, tag="ui")
                                nc.vector.tensor_scalar(
                                    ui[:, 0:512], psa[:], EXPA, EXPB,
                                    Mult, Add)
                                nc.vector.tensor_scalar(
                                    ui[:, 512:1024], psb[:], EXPA, EXPB,
                                    Mult, Add)
                                uv = ui[:].bitcast(f16)
                            else:
                                u = up.tile([128, 1024], f16, tag="u")
                                nc.scalar.activation(u[:, 0:512], psa[:],
                                                     Exp, scale=SCALE)
                                nc.scalar.activation(u[:, 512:1024], psb[:],
                                                     Exp, scale=SCALE)
                                uv = u[:]
                            pending.append((uv, (vas[0], vas[1]), cp, tch, pb))
                            if len(pending) > 2:
                                pv = pending.pop(0)
                                emit_pv(pv)
                                if pv[2] == NCP - 1 and pv[4] == 3:
                                    tail(pv[3])
                            pair_idx += 1
                            # interleave next chunk-pair's kv work
                            target = (n_thunks * pair_idx) // 16
                            while ti < target:
                                nxt[0][ti]()
                                ti += 1
                    while ti < n_thunks:
                        nxt[0][ti]()
                        ti += 1
                    cur = nxt
                for pv in pending:
                    emit_pv(pv)
                    if pv[2] == NCP - 1 and pv[4] == 3:
                        tail(pv[3])

    nc.compile()
    return nc


def _get_nc(reps=1):
    if reps not in _CACHE:
        _CACHE[reps] = _build(reps)
    return _CACHE[reps]


def _pack_act(a, nch):
    """[L, C] row-major -> [L/512, 128, KT, 512] (chunk, partition, k, col)."""
    return np.ascontiguousarray(
        a.reshape(nch, 512, KT, 128).transpose(0, 3, 2, 1))


def _pack_w(w2):
    """[C, 128] -> [128, KT, 128]."""
    return np.ascontiguousarray(w2.reshape(KT, 128, 128).transpose(1, 0, 2))


def _prep_inputs(x, encode_out, Wq, Wk, Wv):
    x = np.asarray(x, dtype=np.float16)
    encode_out = np.asarray(encode_out, dtype=np.float16)
    Wq = np.asarray(Wq, dtype=np.float16)
    Wk = np.asarray(Wk, dtype=np.float16)
    Wv = np.asarray(Wv, dtype=np.float16)
    wqq = _pack_w(np.concatenate([Wq, Wq], axis=1))
    wkv = _pack_w(np.concatenate([Wk, Wv], axis=1))
    wvk = _pack_w(np.concatenate([Wv, Wk], axis=1))
    encTs = [_pack_act(encode_out[b], S // 512) for b in range(B)]
    in_maps = []
    for core in range(NCORE):
        b, th = divmod(core, 2)
        xTi = _pack_act(x[b, th * TSH:(th + 1) * TSH, :], NTCH)
        in_maps.append({"xT": xTi, "encT": encTs[b],
                        "Wqq": wqq, "Wkv": wkv, "Wvk": wvk})
    return in_maps


def kernel(x, encode_out, Wq, Wk, Wv):
    from concourse.bass_utils import run_bass_kernel_spmd
    nc = _get_nc(1)
    in_maps = _prep_inputs(x, encode_out, Wq, Wk, Wv)
    res = run_bass_kernel_spmd(nc, in_maps, list(range(NCORE)))
    out = np.empty((B, T, HS), dtype=np.float32)
    for core in range(NCORE):
        b, th = divmod(core, 2)
        o = res.results[core]["out"]            # [4, 128, 4, 64]
        out[b, th * TSH:(th + 1) * TSH] = (
            o.transpose(0, 2, 1, 3).reshape(TSH, HS))
    return out
